# revision 34
# baseline (speedup 1.0000x reference)
"""Trainium2 Bass kernel for a 3-layer binarized MLP (BNN) with BatchNorm.

Math (reference):
  layer(x, W, a):  y = x_bin @ sign(W).T ; bn = (y - mean)/sqrt(var + eps) over
  the GLOBAL batch; p = prelu(bn, a); out = sign(p) (except last layer).

Key identities used:
  * sign(prelu((y - mu)/std)) == sign(y - mu)   (std > 0, a > 0) -> layers 1,2
    need only the global per-feature mean, not the variance.
  * mean(y) = mean(x_in) @ sign(W).T -> the cross-core all-reduce of the input
    sums can run while the layer's matmuls run.
  * BatchNorm is invariant under positive affine maps of its input, so the
    binarized activations can be recoded {0,1} (b = [y >= mu]) instead of
    {-1,+1}: y_next = 2*(b @ sW) - rowsum(sW) is affine in z = b @ sW, the
    rowsum constants cancel in the mean-threshold comparison, and BN(y)==BN(z)
    for the last layer.  {0,1} signs come from a single DVE is_ge pass.
  * layer 1 splits fp32 x exactly into t1 = fp32r(8192*x) (the scalar
    engine's float32r output rounds to the PE's reduced fp22 precision, so
    the fp32r matmul consumes it losslessly at full bf16 rate) plus the exact
    Sterbenz residual t2' = 8192*x - t1 in fp8e4m3.  The matmul weights are
    pre-scaled by 2^-13, so both terms accumulate into the same PSUM at
    natural scale.  Residual error ~2^-16 relative: inside the sign-flip
    budget (measured final rel err 7.8e-3 vs the 2e-2 gate).
  * layers' fp8 matmuls use DoubleRow (2 contraction tiles per pass, 2x rate);
    all fp8 operand values (0/1/+-1/+-2/+-2^-13 and t2*2^13) are exact.
  * layer 3 (4 output features) packs 4 batch-chunks into the 4 PE column
    groups via tile_position, so PSUM holds [128, 512] = 16 feature-rows and
    all downstream stats/PReLU run at full 128-partition efficiency.

Distribution: pure data-parallel over 8 NeuronCores (batch 65536 -> 8192/core),
weights replicated, 3 tiny AllReduces for the batch statistics.
"""

import sys
import threading

import numpy as np

TRN_REPO = "/opt/trn_rl_repo"
if TRN_REPO not in sys.path:
    sys.path.insert(0, TRN_REPO)

EPS = 1e-5
N_CORES = 8
B = 65536
BC = B // N_CORES          # 8192 rows per core
D0, D1, D2, D3 = 256, 512, 512, 4
NB = 512                   # batch chunk (one PSUM bank of fp32)
NCH = BC // NB             # 16 chunks per core
K1 = D0 // 128             # 2 contraction tiles, layer 1
F1 = D1 // 128             # 4 output tiles, layer 1
K2 = D1 // 128             # 4
F2 = D2 // 128             # 4
K3 = D2 // 128             # 4
GRP = 16                   # phase-T groups (512 rows each)
SUP = 4                    # layer-3 supers (4 chunks col-packed per PSUM bank)

SC13 = 8192.0              # 2^13
INV_SC13 = 1.0 / 8192.0

_LOCK = threading.Lock()
_CACHE = {}


def _build(alpha1, alpha2, alpha3, n_cores=N_CORES, phase=99, dbg=False, reps=1,
           stage1_n=3, stage2_n=8, pstr_bufs=3, ps1_bufs=6, ps2_bufs=5):
    import concourse.bacc as bacc
    import concourse.mybir as mybir
    import concourse.tile as tile
    import concourse.masks as masks

    dt = mybir.dt
    AF = mybir.ActivationFunctionType
    OP = mybir.AluOpType
    AX = mybir.AxisListType
    DR = mybir.MatmulPerfMode.DoubleRow

    nc = bacc.Bacc("TRN2", target_bir_lowering=False, debug=False,
                   num_devices=n_cores)
    x_in = nc.declare_dram_parameter("x", [D0, BC], dt.float32, isOutput=False)
    w1t_in = nc.declare_dram_parameter("w1t", [D0, D1], dt.float32, isOutput=False)
    w2t_in = nc.declare_dram_parameter("w2t", [D1, D2], dt.float32, isOutput=False)
    w3t_in = nc.declare_dram_parameter("w3t", [D2, D3], dt.float32, isOutput=False)
    out_t = nc.declare_dram_parameter("outT", [D3, BC], dt.float32, isOutput=True)

    RG = [list(range(n_cores))]
    inv_b = 1.0 / float(B)

    with tile.TileContext(nc, pool_alloc_mode="queue") as tc:
        with (
            tc.tile_pool(name="w", bufs=1) as pw,
            tc.tile_pool(name="dram", bufs=1, space="DRAM") as pd,
        ):
            for _rep in range(reps):
                # ---------------- int constants (via iota: exact) ---------
                m31_t = pw.tile([128, 1], dt.int32, tag="m31", name="m31")
                nc.gpsimd.iota(m31_t[:], pattern=[[0, 1]], base=31,
                               channel_multiplier=0)
                iota_p = pw.tile([128, 1], dt.int32, tag="iotap", name="iotap")
                nc.gpsimd.iota(iota_p[:], pattern=[[0, 1]], base=0,
                               channel_multiplier=1)
                dvals = pw.tile([128, SUP], dt.int32, tag="dvals", name="dvals")
                nc.gpsimd.iota(dvals[:], pattern=[[1, SUP]], base=0,
                               channel_multiplier=0)
                pm32 = pw.tile([128, 1], dt.int32, tag="pm32", name="pm32")
                nc.vector.tensor_tensor(pm32[:], iota_p[:], m31_t[:],
                                        OP.bitwise_and)
                # maskM[p, d] = 1.0 iff p % 32 == d  (layer-3 partition packing)
                maskM = pw.tile([128, SUP], dt.float32, tag="maskM", name="maskM")
                nc.vector.tensor_tensor(
                    maskM[:], pm32[:, 0:1].broadcast_to((128, SUP)), dvals[:],
                    OP.is_equal)

                # ---------------- weights: load + sign ----------------
                # L1: s1sc fp32 = sign(W1)*2^-13 (fp32r stationary for the t1
                # term); s1dr e5m2 = sign(W1)*2^-13 (DR stationary for t2);
                # s1tf fp32 = sign(W1) for the mean matvec.
                s1tf = [pw.tile([128, D1], dt.float32, tag=f"s1tf{k}",
                                name=f"s1tf{k}") for k in range(K1)]
                s1sc = [pw.tile([128, D1], dt.float32r, tag=f"s1sc{k}",
                                name=f"s1sc{k}") for k in range(K1)]
                s1dr = pw.tile([128, K1 * D1], dt.float8e5, tag="s1dr",
                               name="s1dr")
                # L2: x2 is {0,1}-coded -> weights *2 ; matvec copy also *2.
                s2tf = [pw.tile([128, D2], dt.float32, tag=f"s2tf{k}",
                                name=f"s2tf{k}") for k in range(K2)]
                s2dr = [pw.tile([128, 2 * D2], dt.float8e4, tag=f"s2dr{p}",
                                name=f"s2dr{p}") for p in range(2)]
                # L3: k-tiles 0,1 consume {0,1}-coded x3 -> weights *2;
                # k-tiles 2,3 consume +-1 x3 -> weights *1.  BN is invariant
                # to the resulting affine shift.  (DoubleRow is rejected by
                # the ISA for col-offset tile_position dst, so 4 normal MMs.)
                s3t = [pw.tile([128, D3], dt.float8e4, tag=f"s3t{k}",
                               name=f"s3t{k}") for k in range(K3)]

                cm_wst = tc.tile_pool(name="wst", bufs=3)
                p_wst = cm_wst.__enter__()
                for k in range(K1):
                    wst = p_wst.tile([128, D1], dt.float32, tag="wst", name="wst")
                    nc.sync.dma_start(wst[:], w1t_in[k * 128:(k + 1) * 128, :])
                    nc.scalar.sign(s1tf[k][:], wst[:])
                    nc.scalar.mul(s1sc[k][:], s1tf[k][:], INV_SC13)
                    nc.scalar.mul(s1dr[:, k * D1:(k + 1) * D1], s1tf[k][:],
                                  INV_SC13)
                for k in range(K2):
                    wst = p_wst.tile([128, D2], dt.float32, tag="wst", name="wst")
                    nc.sync.dma_start(wst[:], w2t_in[k * 128:(k + 1) * 128, :])
                    sgn = p_wst.tile([128, D2], dt.float32, tag="sgn", name="sgn")
                    nc.scalar.sign(sgn[:], wst[:])
                    # {0,1} x2 coding -> scale weights by 2
                    nc.scalar.mul(s2tf[k][:], sgn[:], 2.0)
                    nc.scalar.mul(s2dr[k // 2][:, (k % 2) * D2:(k % 2 + 1) * D2],
                                  sgn[:], 2.0)
                for k in range(K3):
                    wst3 = p_wst.tile([128, D3], dt.float32, tag="wst3",
                                      name="wst3")
                    nc.sync.dma_start(wst3[:], w3t_in[k * 128:(k + 1) * 128, :])
                    sc = 2.0 if k < 2 else 1.0
                    sgn3 = p_wst.tile([128, D3], dt.float32, tag="sgn3",
                                      name="sgn3")
                    nc.scalar.sign(sgn3[:], wst3[:])
                    nc.scalar.mul(s3t[k][:], sgn3[:], sc)
                cm_wst.__exit__(None, None, None)

                # ---- persistent activation tiles
                t1a = pw.tile([128, K1 * BC], dt.float32r, tag="t1a",
                              name="t1a")
                t1v = t1a.rearrange("p (k b) -> p k b", k=K1)
                t1s = [t1v[:, k] for k in range(K1)]
                t2p = pw.tile([128, K1 * BC], dt.float8e4, tag="t2p", name="t2p")
                x2p = [pw.tile([128, 2 * BC], dt.float8e4, tag=f"x2p{p}",
                               name=f"x2p{p}") for p in range(2)]
                xsA = pw.tile([128, K1 * GRP], dt.float32, tag="xsA", name="xsA")
                x2s = pw.tile([128, F1 * NCH], dt.float32, tag="x2s", name="x2s")
                thr1 = pw.tile([128, F1], dt.float32, tag="thr1", name="thr1")
                thr2 = pw.tile([128, F2], dt.float32, tag="thr2", name="thr2")
                bias2 = pw.tile([128, F2], dt.float32, tag="bias2", name="bias2")
                y3pk = pw.tile([128, SUP * NB], dt.float16, tag="y3pk",
                               name="y3pk")
                y3st = pw.tile([128, 2 * SUP], dt.float32, tag="y3st",
                               name="y3st")
                sq_scr = pw.tile([128, NB], dt.float32, tag="sqscr",
                                 name="sqscr")

                t2p3 = t2p.rearrange("p (i b) -> p i b", i=2)
                x2p3 = [x2p[p].rearrange("p (i b) -> p i b", i=2)
                        for p in range(2)]
                s1dr3 = s1dr.rearrange("p (i j) -> p i j", i=2)
                s2dr3 = [s2dr[p].rearrange("p (i j) -> p i j", i=2)
                         for p in range(2)]

                # ============ phase T + early L1, interleaved per group ======
                cm_y1st = tc.tile_pool(name="y1st", bufs=1)
                p_y1st = cm_y1st.__enter__()
                y1st = [[p_y1st.tile([128, NB], dt.float32,
                                     tag=f"y1st_{g}_{fo}", name=f"y1st_{g}_{fo}")
                         for fo in range(F1)] for g in range(stage1_n)]

                cm_stage = tc.tile_pool(name="stage", bufs=2)
                p_st = cm_stage.__enter__()
                cm_psA = tc.tile_pool(name="psA", bufs=2, space="PSUM")
                p_psA = cm_psA.__enter__()

                # x arrives pre-transposed from the host ([D0, BC] feature-
                # major), so phase T is pure DMA + elementwise: no PE
                # transposes needed.
                x_r = x_in.rearrange("(k p) b -> p k b", k=K1)

                def phase_t_group(g):
                    xa = p_st.tile([128, K1 * NB], dt.float32, tag="xa",
                                   name="xa")
                    xa3 = xa.rearrange("p (k n) -> p k n", k=K1)
                    gs = slice(g * NB, (g + 1) * NB)
                    nc.sync.dma_start(xa3[:], x_r[:, :, gs])
                    # t1s = fp32r-rounded 8192*x: the scalar engine's
                    # float32r output path performs the same reduced-
                    # precision rounding the PE matmul would apply, so the
                    # t2 residual below is exactly what the fp32r matmul
                    # loses.  One batched op covers both k-tiles.
                    nc.scalar.activation(t1v[:, :, gs], xa3[:], AF.Copy,
                                         scale=SC13)
                    for k in range(K1):
                        src = xa3[:, k]
                        # t2' = 8192*x - t1s  (exact Sterbenz), in e4m3
                        nc.vector.scalar_tensor_tensor(
                            t2p[:, k * BC + g * NB:k * BC + (g + 1) * NB],
                            src, SC13, t1s[k][:, gs].bitcast(dt.float32),
                            op0=OP.mult, op1=OP.subtract)
                    for k in range(K1):
                        # per-feature sum of x for the mean
                        nc.vector.tensor_reduce(
                            xsA[:, k * GRP + g:k * GRP + g + 1], xa3[:, k],
                            axis=AX.X, op=OP.add)

                def l1_mms(cs_list, pts):
                    # fo-major over the chunk list so stationary weights are
                    # reused across consecutive matmuls
                    for fo in range(F1):
                        fsl = slice(fo * 128, (fo + 1) * 128)
                        for c in cs_list:
                            cs = slice(c * NB, (c + 1) * NB)
                            pt = pts[(c, fo)]
                            nc.tensor.matmul(
                                pt[:], s1sc[0][:, fsl], t1s[0][:, cs],
                                start=True, stop=False)
                            nc.tensor.matmul(
                                pt[:], s1sc[1][:, fsl], t1s[1][:, cs],
                                start=False, stop=False)
                            nc.tensor.matmul(
                                pt[:], s1dr3[:, :, fsl], t2p3[:, :, cs],
                                start=False, stop=True, perf_mode=DR)

                def l1_sign(c, src_tiles):
                    # x2b = [y1 >= mu1] in {0,1} e4m3, one DVE pass, with the
                    # per-chunk column sums accumulated for the L2 mean.
                    for fo in range(F1):
                        nc.vector.scalar_tensor_tensor(
                            x2p[fo // 2][:, (fo % 2) * BC + c * NB:
                                         (fo % 2) * BC + (c + 1) * NB],
                            src_tiles[fo][:], 1.0,
                            thr1[:, fo:fo + 1].broadcast_to((128, NB)),
                            op0=OP.mult, op1=OP.is_ge,
                            accum_out=x2s[:, fo * NCH + c:fo * NCH + c + 1])

                for g in range(GRP):
                    phase_t_group(g)
                    if g < stage1_n:
                        pts = {(g, fo): p_psA.tile([128, NB], dt.float32,
                                                   tag="ps1", bufs=ps1_bufs,
                                                   name="ps1")
                               for fo in range(F1)}
                        l1_mms([g], pts)
                        for fo in range(F1):
                            nc.scalar.activation(y1st[g][fo][:],
                                                 pts[(g, fo)][:], AF.Copy)
                cm_stage.__exit__(None, None, None)

                # ---------------- AllReduce #1: sum(x) ----------------
                sumx = pw.tile([128, K1], dt.float32, tag="sumx", name="sumx")
                nc.vector.tensor_reduce(
                    sumx[:], xsA.rearrange("p (k g) -> p k g", k=K1),
                    axis=AX.X, op=OP.add)
                ar1i = pd.tile([128, K1], dt.float32, tag="ar1i", name="ar1i")
                ar1o = pd.tile([128, K1], dt.float32, tag="ar1o", name="ar1o")
                nc.sync.dma_start(ar1i[:], sumx[:])
                nc.gpsimd.collective_compute(
                    "AllReduce", OP.add, replica_groups=RG,
                    ins=[ar1i.opt()], outs=[ar1o.opt()])
                gsumx = pw.tile([128, K1], dt.float32, tag="gsumx", name="gsumx")
                nc.sync.dma_start(gsumx[:], ar1o[:])

                # run-ahead L1 chunk while AllReduce #1 is in flight
                ahead = []
                for c in range(stage1_n, min(stage1_n + 1, NCH)):
                    pts = {(c, fo): p_psA.tile([128, NB], dt.float32,
                                               tag="ps1", bufs=ps1_bufs,
                                               name="ps1")
                           for fo in range(F1)}
                    l1_mms([c], pts)
                    ahead.append((c, pts))

                # mu1 matvec: thr1[:, fo] = (sum(x) @ s1)[fo] / B
                for fo in range(F1):
                    pm = p_psA.tile([128, 1], dt.float32, tag="pm1", bufs=1,
                                    name="pm1")
                    for k in range(K1):
                        nc.tensor.matmul(
                            pm[:], s1tf[k][:, fo * 128:(fo + 1) * 128],
                            gsumx[:, k:k + 1],
                            start=(k == 0), stop=(k == K1 - 1))
                    nc.scalar.mul(thr1[:, fo:fo + 1], pm[:], inv_b)

                # drain staged + run-ahead chunks, then the rest in pairs
                for g in range(stage1_n):
                    l1_sign(g, y1st[g])
                for (c, pts) in ahead:
                    l1_sign(c, {fo: pts[(c, fo)] for fo in range(F1)})
                c0 = stage1_n + len(ahead)
                rest = list(range(c0, NCH))
                i = 0
                while i < len(rest):
                    blk = rest[i:i + 2]
                    # fo-major allocation order matches the matmul issue
                    # order so the ps1 buffer rotation never stalls the PE
                    # behind a sign that hasn't had its inputs yet
                    pts = {(c, fo): p_psA.tile([128, NB], dt.float32,
                                               tag="ps1", bufs=ps1_bufs,
                                               name="ps1")
                           for fo in range(F1) for c in blk}
                    l1_mms(blk, pts)
                    for c in blk:
                        l1_sign(c, {fo: pts[(c, fo)] for fo in range(F1)})
                    i += 2
                cm_psA.__exit__(None, None, None)
                cm_y1st.__exit__(None, None, None)

                # ---------------- AllReduce #2: sum(x2b) ----------------
                x2sum = pw.tile([128, F1], dt.float32, tag="x2sum", name="x2sum")
                nc.vector.tensor_reduce(
                    x2sum[:], x2s.rearrange("p (f c) -> p f c", f=F1),
                    axis=AX.X, op=OP.add)
                ar2i = pd.tile([128, F1], dt.float32, tag="ar2i", name="ar2i")
                ar2o = pd.tile([128, F1], dt.float32, tag="ar2o", name="ar2o")
                nc.sync.dma_start(ar2i[:], x2sum[:])
                nc.gpsimd.collective_compute(
                    "AllReduce", OP.add, replica_groups=RG,
                    ins=[ar2i.opt()], outs=[ar2o.opt()])
                gx2sum = pw.tile([128, K2], dt.float32, tag="gx2sum",
                                 name="gx2sum")
                nc.sync.dma_start(gx2sum[:], ar2o[:])

                # ---------------- layers 2+3 ----------------
                cm_y2st = tc.tile_pool(name="y2st", bufs=1)
                p_y2st = cm_y2st.__enter__()
                y2st = [[p_y2st.tile([128, NB], dt.float16,
                                     tag=f"y2st_{g}_{go}", name=f"y2st_{g}_{go}")
                         for go in range(F2)] for g in range(stage2_n)]
                cm_x3 = tc.tile_pool(name="x3", bufs=3)
                p_x3 = cm_x3.__enter__()
                cm_psB = tc.tile_pool(name="psB", bufs=2, space="PSUM")
                p_psB = cm_psB.__enter__()

                def l2_block_mms(blk):
                    # weight-stationary over the chunk block: one DoubleRow
                    # LDWEIGHTS per (go, pair) amortized over len(blk) matmuls
                    tiles = {}
                    for go in range(F2):
                        gos = slice(go * 128, (go + 1) * 128)
                        for p in range(2):
                            for c in blk:
                                if p == 0:
                                    tiles[(go, c)] = p_psB.tile(
                                        [128, NB], dt.float32, tag="ps2",
                                        bufs=ps2_bufs, name="ps2")
                                cs = slice(c * NB, (c + 1) * NB)
                                nc.tensor.matmul(
                                    tiles[(go, c)][:],
                                    s2dr3[p][:, :, gos], x2p3[p][:, :, cs],
                                    start=(p == 0), stop=(p == 1),
                                    perf_mode=DR)
                    return tiles

                ps3 = {}

                def super_finalize(s):
                    nc.scalar.activation(y3pk[:, s * NB:(s + 1) * NB],
                                         ps3[s][:], AF.Copy)
                    nc.vector.tensor_reduce(
                        y3st[:, s:s + 1], ps3[s][:], axis=AX.X, op=OP.add)
                    nc.vector.tensor_tensor(
                        sq_scr[:], ps3[s][:], y3pk[:, s * NB:(s + 1) * NB],
                        OP.mult)
                    nc.vector.tensor_reduce(
                        y3st[:, SUP + s:SUP + s + 1], sq_scr[:],
                        axis=AX.X, op=OP.add)

                def l23_tail(c, src_tiles):
                    # x3 signs: k-tiles 0,1 as {0,1} on DVE; 2,3 as +-1 on
                    # the scalar engine (weights pre-scaled accordingly; BN
                    # absorbs the affine shift).  Then the col-packed L3
                    # matmuls into ps3[super].
                    x3c = []
                    for go in range(F2):
                        x3t = p_x3.tile([128, NB], dt.float8e4, tag=f"x3_{go}",
                                        bufs=3, name=f"x3_{go}")
                        if go < 2:
                            nc.vector.tensor_scalar(
                                x3t[:], src_tiles[go][:],
                                thr2[:, go:go + 1], None, op0=OP.is_ge)
                        else:
                            nc.scalar.activation(
                                x3t[:], src_tiles[go][:], AF.Sign,
                                bias=bias2[:, go:go + 1])
                        x3c.append(x3t)
                    s, m = c // 4, c % 4
                    if m == 0:
                        ps3[s] = p_psB.tile([128, NB], dt.float32, tag="ps3",
                                            bufs=2, name="ps3")
                        if s < 2:
                            # later supers recycle these banks; their
                            # unwritten partitions keep finite stale values
                            # that maskM zeroes out downstream
                            nc.vector.memset(ps3[s][:], 0.0)
                    for k in range(K3):
                        nc.tensor.matmul(
                            ps3[s][32 * m:32 * m + D3, :],
                            s3t[k][:, 0:D3], x3c[k][:],
                            start=(k == 0), stop=(k == K3 - 1),
                            tile_position=(0, 32 * m))
                    if m == 3:
                        super_finalize(s)

                # staged chunks: blocked mms + copy to y2st (fp16 exact ints)
                for bs in range(0, stage2_n, 4):
                    blk = list(range(bs, min(bs + 4, stage2_n)))
                    tiles = l2_block_mms(blk)
                    for c in blk:
                        for go in range(F2):
                            if go < 2:
                                nc.scalar.activation(y2st[c][go][:],
                                                     tiles[(go, c)][:],
                                                     AF.Copy)
                            else:
                                nc.vector.tensor_copy(y2st[c][go][:],
                                                      tiles[(go, c)][:])

                # run-ahead chunk (fits in the ps2 pool without blocking)
                ra = []
                for c in range(stage2_n, min(stage2_n + 1, NCH)):
                    tiles = l2_block_mms([c])
                    ra.append((c, tiles))

                # mu2 matvec (waits on AllReduce #2).  x2 is {0,1}-coded and
                # s2tf carries the 2x scale; the rowsum shift cancels exactly
                # against the recoding, so the formula is unchanged.
                for go in range(F2):
                    pm = p_psB.tile([128, 1], dt.float32, tag="pm2", bufs=1,
                                    name="pm2")
                    for k in range(K2):
                        nc.tensor.matmul(
                            pm[:], s2tf[k][:, go * 128:(go + 1) * 128],
                            gx2sum[:, k:k + 1],
                            start=(k == 0), stop=(k == K2 - 1))
                    nc.scalar.mul(thr2[:, go:go + 1], pm[:], inv_b)
                    nc.scalar.mul(bias2[:, go:go + 1], pm[:], -inv_b)

                for c in range(stage2_n):
                    l23_tail(c, y2st[c])
                for (c, tiles) in ra:
                    l23_tail(c, {go: tiles[(go, c)] for go in range(F2)})
                c0 = stage2_n + len(ra)
                rest = list(range(c0, NCH))
                i = 0
                while i < len(rest):
                    blk = rest[i:i + 4]
                    tiles = l2_block_mms(blk)
                    for c in blk:
                        l23_tail(c, {go: tiles[(go, c)] for go in range(F2)})
                    i += 4
                cm_x3.__exit__(None, None, None)
                cm_y2st.__exit__(None, None, None)
                cm_psB.__exit__(None, None, None)

                # ---------------- AllReduce #3 + bn3 + prelu -------------
                # combine the col-packed per-super stats across partition
                # groups with a tiny maskM matvec: pm3[d, col] =
                # sum_m y3st[32m+d, col]
                cm_psC = tc.tile_pool(name="psC", bufs=1, space="PSUM")
                p_psC = cm_psC.__enter__()
                pm3 = p_psC.tile([D3, 2 * SUP], dt.float32, tag="pm3",
                                 name="pm3")
                nc.tensor.matmul(pm3[:], maskM[:, 0:D3], y3st[:],
                                 start=True, stop=True)
                st3 = pw.tile([D3, 2], dt.float32, tag="st3", name="st3")
                nc.vector.tensor_reduce(
                    st3[:, 0:1], pm3[:, 0:SUP], axis=AX.X, op=OP.add)
                nc.vector.tensor_reduce(
                    st3[:, 1:2], pm3[:, SUP:2 * SUP], axis=AX.X, op=OP.add)
                ar3i = pd.tile([D3, 2], dt.float32, tag="ar3i", name="ar3i")
                ar3o = pd.tile([D3, 2], dt.float32, tag="ar3o", name="ar3o")
                nc.sync.dma_start(ar3i[:], st3[:])
                nc.gpsimd.collective_compute(
                    "AllReduce", OP.add, replica_groups=RG,
                    ins=[ar3i.opt()], outs=[ar3o.opt()])
                # broadcast [4,2] global stats to all 4 partition groups
                gst3b = pw.tile([128, 2], dt.float32, tag="gst3b", name="gst3b")
                for m in range(4):
                    nc.sync.dma_start(gst3b[32 * m:32 * m + D3, :], ar3o[:])

                mu3 = pw.tile([128, 1], dt.float32, tag="mu3", name="mu3")
                ex2 = pw.tile([128, 1], dt.float32, tag="ex2", name="ex2")
                mu3sq = pw.tile([128, 1], dt.float32, tag="mu3sq", name="mu3sq")
                var3 = pw.tile([128, 1], dt.float32, tag="var3", name="var3")
                epsT = pw.tile([128, 1], dt.float32, tag="epsT", name="epsT")
                vare = pw.tile([128, 1], dt.float32, tag="vare", name="vare")
                rec = pw.tile([128, 1], dt.float32, tag="rec", name="rec")
                scale3 = pw.tile([128, 1], dt.float32, tag="scale3",
                                 name="scale3")
                msc = pw.tile([128, 1], dt.float32, tag="msc", name="msc")
                bias3 = pw.tile([128, 1], dt.float32, tag="bias3", name="bias3")
                nc.scalar.mul(mu3[:], gst3b[:, 0:1], inv_b)
                nc.scalar.mul(ex2[:], gst3b[:, 1:2], inv_b)
                nc.vector.tensor_tensor(mu3sq[:], mu3[:], mu3[:], OP.mult)
                nc.vector.tensor_tensor(var3[:], ex2[:], mu3sq[:], OP.subtract)
                nc.vector.memset(epsT[:], EPS)
                nc.vector.tensor_tensor(vare[:], var3[:], epsT[:], OP.add)
                nc.vector.reciprocal(rec[:], vare[:])
                nc.scalar.sqrt(scale3[:], rec[:])
                nc.vector.tensor_tensor(msc[:], mu3[:], scale3[:], OP.mult)
                nc.scalar.mul(bias3[:], msc[:], -1.0)

                cm_out = tc.tile_pool(name="out", bufs=1)
                p_out = cm_out.__enter__()
                for s in range(SUP):
                    outsb = p_out.tile([128, NB], dt.float32, tag="outsb",
                                       bufs=2, name="outsb")
                    nc.scalar.activation(
                        outsb[:], y3pk[:, s * NB:(s + 1) * NB], AF.Prelu,
                        bias=bias3[:, 0:1], scale=scale3[:, 0:1],
                        alpha=float(alpha3))
                    for m in range(4):
                        c = 4 * s + m
                        nc.sync.dma_start(
                            out_t[:, c * NB:(c + 1) * NB],
                            outsb[32 * m:32 * m + D3, :])
                cm_out.__exit__(None, None, None)
                cm_psC.__exit__(None, None, None)

    nc.compile()
    return nc


def _make_executable(nc):
    """Build a cached jitted shard_map executable for repeated runs
    (mirrors concourse.bass2jax.run_bass_via_pjrt)."""
    import jax
    import concourse.mybir as mybir
    from concourse import bass2jax
    from jax.experimental.shard_map import shard_map
    from jax.sharding import Mesh, PartitionSpec

    bass2jax.install_neuronx_cc_hook()

    partition_name = (nc.partition_id_tensor.name
                      if nc.partition_id_tensor else None)
    in_names, out_names, out_avals, zero_outs = [], [], [], []
    for alloc in nc.m.functions[0].allocations:
        if not isinstance(alloc, mybir.MemoryLocationSet):
            continue
        if not alloc.memorylocations:
            continue
        name = alloc.memorylocations[0].name
        if alloc.kind == "ExternalInput":
            if name != partition_name:
                in_names.append(name)
        elif alloc.kind == "ExternalOutput":
            shape = tuple(alloc.tensor_shape)
            dtype = mybir.dt.np(alloc.dtype)
            out_names.append(name)
            out_avals.append(jax.core.ShapedArray(shape, dtype))
            zero_outs.append(np.zeros(shape, dtype))
    n_params = len(in_names)
    n_outs = len(out_avals)
    all_in_names = list(in_names) + list(out_names)
    if partition_name is not None:
        all_in_names.append(partition_name)
    donate = tuple(range(n_params, n_params + n_outs))

    def _body(*args):
        operands = list(args)
        if partition_name is not None:
            operands.append(bass2jax.partition_id_tensor())
        outs = bass2jax._bass_exec_p.bind(
            *operands,
            out_avals=tuple(out_avals),
            in_names=tuple(all_in_names),
            out_names=tuple(out_names),
            lowering_input_output_aliases=(),
            sim_require_finite=True,
            sim_require_nnan=True,
            nc=nc,
        )
        return tuple(outs)

    devices = jax.devices()[:N_CORES]
    assert len(devices) == N_CORES, f"need {N_CORES} devices, have {len(jax.devices())}"
    mesh = Mesh(np.asarray(devices), ("core",))
    in_specs = (PartitionSpec("core"),) * (n_params + n_outs)
    out_specs = (PartitionSpec("core"),) * n_outs
    sharded = jax.jit(
        shard_map(_body, mesh=mesh, in_specs=in_specs, out_specs=out_specs,
                  check_rep=False),
        donate_argnums=donate, keep_unused=True)
    return sharded, in_names, out_names, out_avals, zero_outs


def _get_exec(alpha1, alpha2, alpha3):
    key = (float(alpha1), float(alpha2), float(alpha3))
    with _LOCK:
        if key not in _CACHE:
            nc = _build(*key)
            _CACHE[key] = _make_executable(nc)
    return _CACHE[key]


def prepare_inputs(x, W1, W2, W3):
    """Host-side sharding / relayout (no arithmetic): batch-shard x,
    transpose weights, replicate them per core."""
    x = np.asarray(x, dtype=np.float32)
    w1t = np.ascontiguousarray(np.asarray(W1, dtype=np.float32).T)
    w2t = np.ascontiguousarray(np.asarray(W2, dtype=np.float32).T)
    w3t = np.ascontiguousarray(np.asarray(W3, dtype=np.float32).T)
    per_core = {
        # feature-major (transposed) per-core shard of x
        "x": [np.ascontiguousarray(x[c * BC:(c + 1) * BC].T)
              for c in range(N_CORES)],
        "w1t": [w1t] * N_CORES,
        "w2t": [w2t] * N_CORES,
        "w3t": [w3t] * N_CORES,
    }
    return per_core


def run_sharded(per_core, exec_pack):
    sharded, in_names, out_names, out_avals, zero_outs = exec_pack
    concat_in = [np.concatenate(per_core[name], axis=0) for name in in_names]
    concat_zero = [np.zeros((N_CORES * z.shape[0],) + z.shape[1:], z.dtype)
                   for z in zero_outs]
    out_arrs = sharded(*concat_in, *concat_zero)
    outs = {}
    for i, name in enumerate(out_names):
        full = np.asarray(out_arrs[i]).reshape(
            (N_CORES,) + tuple(out_avals[i].shape))
        outs[name] = full
    return outs


def kernel(x, W1, W2, W3, a1, a2, a3):
    exec_pack = _get_exec(float(a1), float(a2), float(a3))
    per_core = prepare_inputs(x, W1, W2, W3)
    outs = run_sharded(per_core, exec_pack)
    out_t = outs["outT"]                     # [N_CORES, 4, BC]
    out = np.empty((B, D3), dtype=np.float32)
    for c in range(N_CORES):
        out[c * BC:(c + 1) * BC] = out_t[c].T
    return out



# revision 37
# speedup vs baseline: 1.1146x; 1.1146x over previous
"""Trainium2 Bass kernel for a 3-layer binarized MLP (BNN) with BatchNorm.

Math (reference):
  layer(x, W, a):  y = x_bin @ sign(W).T ; bn = (y - mean)/sqrt(var + eps) over
  the GLOBAL batch; p = prelu(bn, a); out = sign(p) (except last layer).

Key identities used:
  * sign(prelu((y - mu)/std)) == sign(y - mu)   (std > 0, a > 0) -> layers 1,2
    need only the global per-feature mean, not the variance.
  * mean(y) = mean(x_in) @ sign(W).T -> the cross-core all-reduce of the input
    sums can run while the layer's matmuls run.
  * BatchNorm is invariant under positive affine maps of its input, so the
    binarized activations can be recoded {0,1} (b = [y >= mu]) instead of
    {-1,+1}: y_next = 2*(b @ sW) - rowsum(sW) is affine in z = b @ sW, the
    rowsum constants cancel in the mean-threshold comparison, and BN(y)==BN(z)
    for the last layer.  {0,1} signs come from a single DVE is_ge pass.
  * layer 1 splits fp32 x exactly into t1 = fp32r(8192*x) (the scalar
    engine's float32r output rounds to the PE's reduced fp22 precision, so
    the fp32r matmul consumes it losslessly at full bf16 rate) plus the exact
    Sterbenz residual t2' = 8192*x - t1 in fp8e4m3.  The matmul weights are
    pre-scaled by 2^-13, so both terms accumulate into the same PSUM at
    natural scale.  Residual error ~2^-16 relative: inside the sign-flip
    budget (measured final rel err 7.8e-3 vs the 2e-2 gate).
  * layers' fp8 matmuls use DoubleRow (2 contraction tiles per pass, 2x rate);
    all fp8 operand values (0/1/+-1/+-2/+-2^-13 and t2*2^13) are exact.
  * layer 3 (4 output features) packs 4 batch-chunks into the 4 PE column
    groups via tile_position, so PSUM holds [128, 512] = 16 feature-rows and
    all downstream stats/PReLU run at full 128-partition efficiency.
  * x is transposed on the host (layout-only prep, like the weight
    transposes), so phase T needs no PE transposes at all: the feature-major
    shard DMAs straight into SBUF and t1/t2/means are pure ACT/DVE work.

Distribution: pure data-parallel over 8 NeuronCores (batch 65536 -> 8192/core),
weights replicated, 3 tiny AllReduces for the batch statistics.
"""

import sys
import threading

import numpy as np

TRN_REPO = "/opt/trn_rl_repo"
if TRN_REPO not in sys.path:
    sys.path.insert(0, TRN_REPO)

EPS = 1e-5
N_CORES = 8
B = 65536
BC = B // N_CORES          # 8192 rows per core
D0, D1, D2, D3 = 256, 512, 512, 4
NB = 512                   # batch chunk (one PSUM bank of fp32)
NCH = BC // NB             # 16 chunks per core
K1 = D0 // 128             # 2 contraction tiles, layer 1
F1 = D1 // 128             # 4 output tiles, layer 1
K2 = D1 // 128             # 4
F2 = D2 // 128             # 4
K3 = D2 // 128             # 4
GRP = 16                   # phase-T groups (512 rows each)
SUP = 4                    # layer-3 supers (4 chunks col-packed per PSUM bank)

SC13 = 8192.0              # 2^13
INV_SC13 = 1.0 / 8192.0

_LOCK = threading.Lock()
_CACHE = {}


def _build(alpha1, alpha2, alpha3, n_cores=N_CORES, phase=99, dbg=False, reps=1,
           stage1_n=3, stage2_n=8, pstr_bufs=3, ps1_bufs=4, ps2_bufs=5):
    import concourse.bacc as bacc
    import concourse.mybir as mybir
    import concourse.tile as tile
    import concourse.masks as masks

    dt = mybir.dt
    AF = mybir.ActivationFunctionType
    OP = mybir.AluOpType
    AX = mybir.AxisListType
    DR = mybir.MatmulPerfMode.DoubleRow

    nc = bacc.Bacc("TRN2", target_bir_lowering=False, debug=False,
                   num_devices=n_cores)
    x_in = nc.declare_dram_parameter("x", [D0, BC], dt.float32, isOutput=False)
    w1t_in = nc.declare_dram_parameter("w1t", [D0, D1], dt.float32, isOutput=False)
    w2t_in = nc.declare_dram_parameter("w2t", [D1, D2], dt.float32, isOutput=False)
    w3t_in = nc.declare_dram_parameter("w3t", [D2, D3], dt.float32, isOutput=False)
    out_t = nc.declare_dram_parameter("outT", [D3, BC], dt.float32, isOutput=True)

    RG = [list(range(n_cores))]
    inv_b = 1.0 / float(B)

    with tile.TileContext(nc, pool_alloc_mode="queue") as tc:
        with (
            tc.tile_pool(name="w", bufs=1) as pw,
            tc.tile_pool(name="dram", bufs=1, space="DRAM") as pd,
        ):
            for _rep in range(reps):
                # ---------------- int constants (via iota: exact) ---------
                m31_t = pw.tile([128, 1], dt.int32, tag="m31", name="m31")
                nc.gpsimd.iota(m31_t[:], pattern=[[0, 1]], base=31,
                               channel_multiplier=0)
                iota_p = pw.tile([128, 1], dt.int32, tag="iotap", name="iotap")
                nc.gpsimd.iota(iota_p[:], pattern=[[0, 1]], base=0,
                               channel_multiplier=1)
                dvals = pw.tile([128, SUP], dt.int32, tag="dvals", name="dvals")
                nc.gpsimd.iota(dvals[:], pattern=[[1, SUP]], base=0,
                               channel_multiplier=0)
                pm32 = pw.tile([128, 1], dt.int32, tag="pm32", name="pm32")
                nc.vector.tensor_tensor(pm32[:], iota_p[:], m31_t[:],
                                        OP.bitwise_and)
                # maskM[p, d] = 1.0 iff p % 32 == d  (layer-3 partition packing)
                maskM = pw.tile([128, SUP], dt.float32, tag="maskM", name="maskM")
                nc.vector.tensor_tensor(
                    maskM[:], pm32[:, 0:1].broadcast_to((128, SUP)), dvals[:],
                    OP.is_equal)

                # ---------------- weights: load + sign ----------------
                # L1: s1sc fp32 = sign(W1)*2^-13 (fp32r stationary for the t1
                # term); s1dr e5m2 = sign(W1)*2^-13 (DR stationary for t2);
                # s1tf fp32 = sign(W1) for the mean matvec.
                s1tf = [pw.tile([128, D1], dt.float32, tag=f"s1tf{k}",
                                name=f"s1tf{k}") for k in range(K1)]
                s1sc = [pw.tile([128, D1], dt.float32r, tag=f"s1sc{k}",
                                name=f"s1sc{k}") for k in range(K1)]
                s1dr = pw.tile([128, K1 * D1], dt.float8e5, tag="s1dr",
                               name="s1dr")
                # L2: x2 is {0,1}-coded -> weights *2 ; matvec copy also *2.
                s2tf = [pw.tile([128, D2], dt.float32, tag=f"s2tf{k}",
                                name=f"s2tf{k}") for k in range(K2)]
                s2dr = [pw.tile([128, 2 * D2], dt.float8e4, tag=f"s2dr{p}",
                                name=f"s2dr{p}") for p in range(2)]
                # L3: k-tiles 0,1 consume {0,1}-coded x3 -> weights *2;
                # k-tiles 2,3 consume +-1 x3 -> weights *1.  BN is invariant
                # to the resulting affine shift.  (DoubleRow is rejected by
                # the ISA for col-offset tile_position dst, so 4 normal MMs.)
                s3t = [pw.tile([128, D3], dt.float8e4, tag=f"s3t{k}",
                               name=f"s3t{k}") for k in range(K3)]

                cm_wst = tc.tile_pool(name="wst", bufs=3)
                p_wst = cm_wst.__enter__()
                for k in range(K1):
                    wst = p_wst.tile([128, D1], dt.float32, tag="wst", name="wst")
                    nc.sync.dma_start(wst[:], w1t_in[k * 128:(k + 1) * 128, :])
                    nc.scalar.sign(s1tf[k][:], wst[:])
                    nc.scalar.mul(s1sc[k][:], s1tf[k][:], INV_SC13)
                    nc.scalar.mul(s1dr[:, k * D1:(k + 1) * D1], s1tf[k][:],
                                  INV_SC13)
                for k in range(K2):
                    wst = p_wst.tile([128, D2], dt.float32, tag="wst", name="wst")
                    nc.sync.dma_start(wst[:], w2t_in[k * 128:(k + 1) * 128, :])
                    sgn = p_wst.tile([128, D2], dt.float32, tag="sgn", name="sgn")
                    nc.scalar.sign(sgn[:], wst[:])
                    # {0,1} x2 coding -> scale weights by 2
                    nc.scalar.mul(s2tf[k][:], sgn[:], 2.0)
                    nc.scalar.mul(s2dr[k // 2][:, (k % 2) * D2:(k % 2 + 1) * D2],
                                  sgn[:], 2.0)
                for k in range(K3):
                    wst3 = p_wst.tile([128, D3], dt.float32, tag="wst3",
                                      name="wst3")
                    nc.sync.dma_start(wst3[:], w3t_in[k * 128:(k + 1) * 128, :])
                    sc = 2.0 if k < 2 else 1.0
                    sgn3 = p_wst.tile([128, D3], dt.float32, tag="sgn3",
                                      name="sgn3")
                    nc.scalar.sign(sgn3[:], wst3[:])
                    nc.scalar.mul(s3t[k][:], sgn3[:], sc)
                cm_wst.__exit__(None, None, None)

                # ---- persistent activation tiles
                t1s = [pw.tile([128, BC], dt.float32r, tag=f"t1s{k}",
                               name=f"t1s{k}") for k in range(K1)]
                t2p = pw.tile([128, K1 * BC], dt.float8e4, tag="t2p", name="t2p")
                x2p = [pw.tile([128, 2 * BC], dt.float8e4, tag=f"x2p{p}",
                               name=f"x2p{p}") for p in range(2)]
                xsA = pw.tile([128, K1 * GRP], dt.float32, tag="xsA", name="xsA")
                x2s = pw.tile([128, F1 * NCH], dt.float32, tag="x2s", name="x2s")
                thr1 = pw.tile([128, F1], dt.float32, tag="thr1", name="thr1")
                thr2 = pw.tile([128, F2], dt.float32, tag="thr2", name="thr2")
                bias2 = pw.tile([128, F2], dt.float32, tag="bias2", name="bias2")
                y3pk = pw.tile([128, SUP * NB], dt.float16, tag="y3pk",
                               name="y3pk")
                y3st = pw.tile([128, 2 * SUP], dt.float32, tag="y3st",
                               name="y3st")
                sq_scr = pw.tile([128, NB], dt.float32, tag="sqscr",
                                 name="sqscr")

                t2p3 = t2p.rearrange("p (i b) -> p i b", i=2)
                x2p3 = [x2p[p].rearrange("p (i b) -> p i b", i=2)
                        for p in range(2)]
                s1dr3 = s1dr.rearrange("p (i j) -> p i j", i=2)
                s2dr3 = [s2dr[p].rearrange("p (i j) -> p i j", i=2)
                         for p in range(2)]

                # ============ phase T + early L1, interleaved per group ======
                cm_y1st = tc.tile_pool(name="y1st", bufs=1)
                p_y1st = cm_y1st.__enter__()
                y1st = [[p_y1st.tile([128, NB], dt.float32,
                                     tag=f"y1st_{g}_{fo}", name=f"y1st_{g}_{fo}")
                         for fo in range(F1)] for g in range(stage1_n)]

                cm_stage = tc.tile_pool(name="stage", bufs=2)
                p_st = cm_stage.__enter__()
                cm_psA = tc.tile_pool(name="psA", bufs=2, space="PSUM")
                p_psA = cm_psA.__enter__()

                # x arrives pre-transposed from the host ([D0, BC] feature-
                # major), so phase T is pure DMA + elementwise: no PE
                # transposes needed.
                x_r = x_in.rearrange("(k p) b -> p k b", k=K1)

                def phase_t_group(g):
                    xa = p_st.tile([128, K1 * NB], dt.float32, tag="xa",
                                   name="xa")
                    xa3 = xa.rearrange("p (k n) -> p k n", k=K1)
                    gs = slice(g * NB, (g + 1) * NB)
                    nc.sync.dma_start(xa3[:], x_r[:, :, gs])
                    for k in range(K1):
                        src = xa3[:, k]
                        # t1s = fp32r-rounded 8192*x: the scalar engine's
                        # float32r output path performs the same reduced-
                        # precision rounding the PE matmul would apply, so
                        # the t2 residual below is exactly what the fp32r
                        # matmul loses.
                        nc.scalar.activation(
                            t1s[k][:, gs], src, AF.Copy, scale=SC13)
                        # t2' = 8192*x - t1s  (exact Sterbenz), in e4m3
                        nc.vector.scalar_tensor_tensor(
                            t2p[:, k * BC + g * NB:k * BC + (g + 1) * NB],
                            src, SC13, t1s[k][:, gs].bitcast(dt.float32),
                            op0=OP.mult, op1=OP.subtract)
                        # per-feature sum of x for the mean
                        nc.vector.tensor_reduce(
                            xsA[:, k * GRP + g:k * GRP + g + 1], src,
                            axis=AX.X, op=OP.add)

                def l1_mms(cs_list, pts):
                    # fo-major over the chunk list so stationary weights are
                    # reused across consecutive matmuls
                    for fo in range(F1):
                        fsl = slice(fo * 128, (fo + 1) * 128)
                        for c in cs_list:
                            cs = slice(c * NB, (c + 1) * NB)
                            pt = pts[(c, fo)]
                            nc.tensor.matmul(
                                pt[:], s1sc[0][:, fsl], t1s[0][:, cs],
                                start=True, stop=False)
                            nc.tensor.matmul(
                                pt[:], s1sc[1][:, fsl], t1s[1][:, cs],
                                start=False, stop=False)
                            nc.tensor.matmul(
                                pt[:], s1dr3[:, :, fsl], t2p3[:, :, cs],
                                start=False, stop=True, perf_mode=DR)

                def l1_sign(c, src_tiles):
                    # x2b = [y1 >= mu1] in {0,1} e4m3, one DVE pass, with the
                    # per-chunk column sums accumulated for the L2 mean.
                    cs = slice(c * NB, (c + 1) * NB)
                    for fo in range(F1):
                        nc.vector.scalar_tensor_tensor(
                            x2p[fo // 2][:, (fo % 2) * BC + c * NB:
                                         (fo % 2) * BC + (c + 1) * NB],
                            src_tiles[fo][:], 1.0,
                            thr1[:, fo:fo + 1].broadcast_to((128, NB)),
                            op0=OP.mult, op1=OP.is_ge,
                            accum_out=x2s[:, fo * NCH + c:fo * NCH + c + 1])

                for g in range(GRP):
                    phase_t_group(g)
                    if g < stage1_n:
                        pts = {(g, fo): p_psA.tile([128, NB], dt.float32,
                                                   tag="ps1", bufs=ps1_bufs,
                                                   name="ps1")
                               for fo in range(F1)}
                        l1_mms([g], pts)
                        for fo in range(F1):
                            nc.scalar.activation(y1st[g][fo][:],
                                                 pts[(g, fo)][:], AF.Copy)
                cm_stage.__exit__(None, None, None)

                # ---------------- AllReduce #1: sum(x) ----------------
                sumx = pw.tile([128, K1], dt.float32, tag="sumx", name="sumx")
                nc.vector.tensor_reduce(
                    sumx[:], xsA.rearrange("p (k g) -> p k g", k=K1),
                    axis=AX.X, op=OP.add)
                ar1i = pd.tile([128, K1], dt.float32, tag="ar1i", name="ar1i")
                ar1o = pd.tile([128, K1], dt.float32, tag="ar1o", name="ar1o")
                nc.sync.dma_start(ar1i[:], sumx[:])
                nc.gpsimd.collective_compute(
                    "AllReduce", OP.add, replica_groups=RG,
                    ins=[ar1i.opt()], outs=[ar1o.opt()])
                gsumx = pw.tile([128, K1], dt.float32, tag="gsumx", name="gsumx")
                nc.sync.dma_start(gsumx[:], ar1o[:])

                # run-ahead L1 chunk while AllReduce #1 is in flight
                ahead = []
                for c in range(stage1_n, min(stage1_n + 1, NCH)):
                    pts = {(c, fo): p_psA.tile([128, NB], dt.float32,
                                               tag="ps1", bufs=ps1_bufs,
                                               name="ps1")
                           for fo in range(F1)}
                    l1_mms([c], pts)
                    ahead.append((c, pts))

                # mu1 matvec: thr1[:, fo] = (sum(x) @ s1)[fo] / B
                for fo in range(F1):
                    pm = p_psA.tile([128, 1], dt.float32, tag="pm1", bufs=1,
                                    name="pm1")
                    for k in range(K1):
                        nc.tensor.matmul(
                            pm[:], s1tf[k][:, fo * 128:(fo + 1) * 128],
                            gsumx[:, k:k + 1],
                            start=(k == 0), stop=(k == K1 - 1))
                    nc.scalar.mul(thr1[:, fo:fo + 1], pm[:], inv_b)

                # drain staged + run-ahead chunks, then the rest in pairs
                for g in range(stage1_n):
                    l1_sign(g, y1st[g])
                for (c, pts) in ahead:
                    l1_sign(c, {fo: pts[(c, fo)] for fo in range(F1)})
                c0 = stage1_n + len(ahead)
                rest = list(range(c0, NCH))
                i = 0
                while i < len(rest):
                    blk = rest[i:i + 2]
                    # fo-major allocation order matches the matmul issue
                    # order so the ps1 buffer rotation never stalls the PE
                    # behind a sign that hasn't had its inputs yet
                    pts = {(c, fo): p_psA.tile([128, NB], dt.float32,
                                               tag="ps1", bufs=ps1_bufs,
                                               name="ps1")
                           for fo in range(F1) for c in blk}
                    l1_mms(blk, pts)
                    for c in blk:
                        l1_sign(c, {fo: pts[(c, fo)] for fo in range(F1)})
                    i += 2
                cm_psA.__exit__(None, None, None)
                cm_y1st.__exit__(None, None, None)

                # ---------------- AllReduce #2: sum(x2b) ----------------
                x2sum = pw.tile([128, F1], dt.float32, tag="x2sum", name="x2sum")
                nc.vector.tensor_reduce(
                    x2sum[:], x2s.rearrange("p (f c) -> p f c", f=F1),
                    axis=AX.X, op=OP.add)
                ar2i = pd.tile([128, F1], dt.float32, tag="ar2i", name="ar2i")
                ar2o = pd.tile([128, F1], dt.float32, tag="ar2o", name="ar2o")
                nc.sync.dma_start(ar2i[:], x2sum[:])
                nc.gpsimd.collective_compute(
                    "AllReduce", OP.add, replica_groups=RG,
                    ins=[ar2i.opt()], outs=[ar2o.opt()])
                gx2sum = pw.tile([128, K2], dt.float32, tag="gx2sum",
                                 name="gx2sum")
                nc.sync.dma_start(gx2sum[:], ar2o[:])

                # ---------------- layers 2+3 ----------------
                cm_y2st = tc.tile_pool(name="y2st", bufs=1)
                p_y2st = cm_y2st.__enter__()
                y2st = [[p_y2st.tile([128, NB], dt.float16,
                                     tag=f"y2st_{g}_{go}", name=f"y2st_{g}_{go}")
                         for go in range(F2)] for g in range(stage2_n)]
                cm_x3 = tc.tile_pool(name="x3", bufs=3)
                p_x3 = cm_x3.__enter__()
                cm_psB = tc.tile_pool(name="psB", bufs=2, space="PSUM")
                p_psB = cm_psB.__enter__()

                def l2_block_mms(blk):
                    # weight-stationary over the chunk block: one DoubleRow
                    # LDWEIGHTS per (go, pair) amortized over len(blk) matmuls
                    tiles = {}
                    for go in range(F2):
                        gos = slice(go * 128, (go + 1) * 128)
                        for p in range(2):
                            for c in blk:
                                if p == 0:
                                    tiles[(go, c)] = p_psB.tile(
                                        [128, NB], dt.float32, tag="ps2",
                                        bufs=ps2_bufs, name="ps2")
                                cs = slice(c * NB, (c + 1) * NB)
                                nc.tensor.matmul(
                                    tiles[(go, c)][:],
                                    s2dr3[p][:, :, gos], x2p3[p][:, :, cs],
                                    start=(p == 0), stop=(p == 1),
                                    perf_mode=DR)
                    return tiles

                ps3 = {}

                def super_finalize(s):
                    nc.scalar.activation(y3pk[:, s * NB:(s + 1) * NB],
                                         ps3[s][:], AF.Copy)
                    nc.vector.tensor_reduce(
                        y3st[:, s:s + 1], ps3[s][:], axis=AX.X, op=OP.add)
                    nc.vector.tensor_tensor(
                        sq_scr[:], ps3[s][:], y3pk[:, s * NB:(s + 1) * NB],
                        OP.mult)
                    nc.vector.tensor_reduce(
                        y3st[:, SUP + s:SUP + s + 1], sq_scr[:],
                        axis=AX.X, op=OP.add)

                def l23_tail(c, src_tiles):
                    # x3 signs: k-tiles 0,1 as {0,1} on DVE; 2,3 as +-1 on
                    # the scalar engine (weights pre-scaled accordingly; BN
                    # absorbs the affine shift).  Then the col-packed L3
                    # matmuls into ps3[super].
                    x3c = []
                    for go in range(F2):
                        x3t = p_x3.tile([128, NB], dt.float8e4, tag=f"x3_{go}",
                                        bufs=3, name=f"x3_{go}")
                        if go < 2:
                            nc.vector.scalar_tensor_tensor(
                                x3t[:], src_tiles[go][:], 1.0,
                                thr2[:, go:go + 1].broadcast_to((128, NB)),
                                op0=OP.mult, op1=OP.is_ge)
                        else:
                            nc.scalar.activation(
                                x3t[:], src_tiles[go][:], AF.Sign,
                                bias=bias2[:, go:go + 1])
                        x3c.append(x3t)
                    s, m = c // 4, c % 4
                    if m == 0:
                        ps3[s] = p_psB.tile([128, NB], dt.float32, tag="ps3",
                                            bufs=2, name="ps3")
                        nc.vector.memset(ps3[s][:], 0.0)
                    for k in range(K3):
                        nc.tensor.matmul(
                            ps3[s][32 * m:32 * m + D3, :],
                            s3t[k][:, 0:D3], x3c[k][:],
                            start=(k == 0), stop=(k == K3 - 1),
                            tile_position=(0, 32 * m))
                    if m == 3:
                        super_finalize(s)

                # staged chunks: blocked mms + copy to y2st (fp16 exact ints)
                for bs in range(0, stage2_n, 4):
                    blk = list(range(bs, min(bs + 4, stage2_n)))
                    tiles = l2_block_mms(blk)
                    for c in blk:
                        for go in range(F2):
                            if go < 2:
                                nc.scalar.activation(y2st[c][go][:],
                                                     tiles[(go, c)][:],
                                                     AF.Copy)
                            else:
                                nc.vector.tensor_copy(y2st[c][go][:],
                                                      tiles[(go, c)][:])

                # run-ahead chunk (fits in the ps2 pool without blocking)
                ra = []
                for c in range(stage2_n, min(stage2_n + 1, NCH)):
                    tiles = l2_block_mms([c])
                    ra.append((c, tiles))

                # mu2 matvec (waits on AllReduce #2).  x2 is {0,1}-coded and
                # s2tf carries the 2x scale; the rowsum shift cancels exactly
                # against the recoding, so the formula is unchanged.
                for go in range(F2):
                    pm = p_psB.tile([128, 1], dt.float32, tag="pm2", bufs=1,
                                    name="pm2")
                    for k in range(K2):
                        nc.tensor.matmul(
                            pm[:], s2tf[k][:, go * 128:(go + 1) * 128],
                            gx2sum[:, k:k + 1],
                            start=(k == 0), stop=(k == K2 - 1))
                    nc.scalar.mul(thr2[:, go:go + 1], pm[:], inv_b)
                    nc.scalar.mul(bias2[:, go:go + 1], pm[:], -inv_b)

                for c in range(stage2_n):
                    l23_tail(c, y2st[c])
                for (c, tiles) in ra:
                    l23_tail(c, {go: tiles[(go, c)] for go in range(F2)})
                c0 = stage2_n + len(ra)
                rest = list(range(c0, NCH))
                i = 0
                while i < len(rest):
                    blk = rest[i:i + 4]
                    tiles = l2_block_mms(blk)
                    for c in blk:
                        l23_tail(c, {go: tiles[(go, c)] for go in range(F2)})
                    i += 4
                cm_x3.__exit__(None, None, None)
                cm_y2st.__exit__(None, None, None)
                cm_psB.__exit__(None, None, None)

                # ---------------- AllReduce #3 + bn3 + prelu -------------
                # combine the col-packed per-super stats across partition
                # groups with a tiny maskM matvec: pm3[d, col] =
                # sum_m y3st[32m+d, col]
                cm_psC = tc.tile_pool(name="psC", bufs=1, space="PSUM")
                p_psC = cm_psC.__enter__()
                pm3 = p_psC.tile([D3, 2 * SUP], dt.float32, tag="pm3",
                                 name="pm3")
                nc.tensor.matmul(pm3[:], maskM[:, 0:D3], y3st[:],
                                 start=True, stop=True)
                st3 = pw.tile([D3, 2], dt.float32, tag="st3", name="st3")
                nc.vector.tensor_reduce(
                    st3[:, 0:1], pm3[:, 0:SUP], axis=AX.X, op=OP.add)
                nc.vector.tensor_reduce(
                    st3[:, 1:2], pm3[:, SUP:2 * SUP], axis=AX.X, op=OP.add)
                ar3i = pd.tile([D3, 2], dt.float32, tag="ar3i", name="ar3i")
                ar3o = pd.tile([D3, 2], dt.float32, tag="ar3o", name="ar3o")
                nc.sync.dma_start(ar3i[:], st3[:])
                nc.gpsimd.collective_compute(
                    "AllReduce", OP.add, replica_groups=RG,
                    ins=[ar3i.opt()], outs=[ar3o.opt()])
                # broadcast [4,2] global stats to all 4 partition groups
                gst3b = pw.tile([128, 2], dt.float32, tag="gst3b", name="gst3b")
                for m in range(4):
                    nc.sync.dma_start(gst3b[32 * m:32 * m + D3, :], ar3o[:])

                mu3 = pw.tile([128, 1], dt.float32, tag="mu3", name="mu3")
                ex2 = pw.tile([128, 1], dt.float32, tag="ex2", name="ex2")
                mu3sq = pw.tile([128, 1], dt.float32, tag="mu3sq", name="mu3sq")
                var3 = pw.tile([128, 1], dt.float32, tag="var3", name="var3")
                epsT = pw.tile([128, 1], dt.float32, tag="epsT", name="epsT")
                vare = pw.tile([128, 1], dt.float32, tag="vare", name="vare")
                rec = pw.tile([128, 1], dt.float32, tag="rec", name="rec")
                scale3 = pw.tile([128, 1], dt.float32, tag="scale3",
                                 name="scale3")
                msc = pw.tile([128, 1], dt.float32, tag="msc", name="msc")
                bias3 = pw.tile([128, 1], dt.float32, tag="bias3", name="bias3")
                nc.scalar.mul(mu3[:], gst3b[:, 0:1], inv_b)
                nc.scalar.mul(ex2[:], gst3b[:, 1:2], inv_b)
                nc.vector.tensor_tensor(mu3sq[:], mu3[:], mu3[:], OP.mult)
                nc.vector.tensor_tensor(var3[:], ex2[:], mu3sq[:], OP.subtract)
                nc.vector.memset(epsT[:], EPS)
                nc.vector.tensor_tensor(vare[:], var3[:], epsT[:], OP.add)
                nc.vector.reciprocal(rec[:], vare[:])
                nc.scalar.sqrt(scale3[:], rec[:])
                nc.vector.tensor_tensor(msc[:], mu3[:], scale3[:], OP.mult)
                nc.scalar.mul(bias3[:], msc[:], -1.0)

                cm_out = tc.tile_pool(name="out", bufs=1)
                p_out = cm_out.__enter__()
                for s in range(SUP):
                    outsb = p_out.tile([128, NB], dt.float32, tag="outsb",
                                       bufs=2, name="outsb")
                    nc.scalar.activation(
                        outsb[:], y3pk[:, s * NB:(s + 1) * NB], AF.Prelu,
                        bias=bias3[:, 0:1], scale=scale3[:, 0:1],
                        alpha=float(alpha3))
                    for m in range(4):
                        c = 4 * s + m
                        nc.sync.dma_start(
                            out_t[:, c * NB:(c + 1) * NB],
                            outsb[32 * m:32 * m + D3, :])
                cm_out.__exit__(None, None, None)
                cm_psC.__exit__(None, None, None)

    nc.compile()
    return nc


def _make_executable(nc):
    """Build a cached jitted shard_map executable for repeated runs
    (mirrors concourse.bass2jax.run_bass_via_pjrt)."""
    import jax
    import concourse.mybir as mybir
    from concourse import bass2jax
    from jax.experimental.shard_map import shard_map
    from jax.sharding import Mesh, PartitionSpec

    bass2jax.install_neuronx_cc_hook()

    partition_name = (nc.partition_id_tensor.name
                      if nc.partition_id_tensor else None)
    in_names, out_names, out_avals, zero_outs = [], [], [], []
    for alloc in nc.m.functions[0].allocations:
        if not isinstance(alloc, mybir.MemoryLocationSet):
            continue
        if not alloc.memorylocations:
            continue
        name = alloc.memorylocations[0].name
        if alloc.kind == "ExternalInput":
            if name != partition_name:
                in_names.append(name)
        elif alloc.kind == "ExternalOutput":
            shape = tuple(alloc.tensor_shape)
            dtype = mybir.dt.np(alloc.dtype)
            out_names.append(name)
            out_avals.append(jax.core.ShapedArray(shape, dtype))
            zero_outs.append(np.zeros(shape, dtype))
    n_params = len(in_names)
    n_outs = len(out_avals)
    all_in_names = list(in_names) + list(out_names)
    if partition_name is not None:
        all_in_names.append(partition_name)
    donate = tuple(range(n_params, n_params + n_outs))

    def _body(*args):
        operands = list(args)
        if partition_name is not None:
            operands.append(bass2jax.partition_id_tensor())
        outs = bass2jax._bass_exec_p.bind(
            *operands,
            out_avals=tuple(out_avals),
            in_names=tuple(all_in_names),
            out_names=tuple(out_names),
            lowering_input_output_aliases=(),
            sim_require_finite=True,
            sim_require_nnan=True,
            nc=nc,
        )
        return tuple(outs)

    devices = jax.devices()[:N_CORES]
    assert len(devices) == N_CORES, f"need {N_CORES} devices, have {len(jax.devices())}"
    mesh = Mesh(np.asarray(devices), ("core",))
    in_specs = (PartitionSpec("core"),) * (n_params + n_outs)
    out_specs = (PartitionSpec("core"),) * n_outs
    sharded = jax.jit(
        shard_map(_body, mesh=mesh, in_specs=in_specs, out_specs=out_specs,
                  check_rep=False),
        donate_argnums=donate, keep_unused=True)
    return sharded, in_names, out_names, out_avals, zero_outs


def _get_exec(alpha1, alpha2, alpha3):
    key = (float(alpha1), float(alpha2), float(alpha3))
    with _LOCK:
        if key not in _CACHE:
            nc = _build(*key)
            _CACHE[key] = _make_executable(nc)
    return _CACHE[key]


def prepare_inputs(x, W1, W2, W3):
    """Host-side sharding / relayout (no arithmetic): batch-shard x,
    transpose weights, replicate them per core."""
    x = np.asarray(x, dtype=np.float32)
    w1t = np.ascontiguousarray(np.asarray(W1, dtype=np.float32).T)
    w2t = np.ascontiguousarray(np.asarray(W2, dtype=np.float32).T)
    w3t = np.ascontiguousarray(np.asarray(W3, dtype=np.float32).T)
    per_core = {
        # feature-major (transposed) per-core shard of x
        "x": [np.ascontiguousarray(x[c * BC:(c + 1) * BC].T)
              for c in range(N_CORES)],
        "w1t": [w1t] * N_CORES,
        "w2t": [w2t] * N_CORES,
        "w3t": [w3t] * N_CORES,
    }
    return per_core


def run_sharded(per_core, exec_pack):
    sharded, in_names, out_names, out_avals, zero_outs = exec_pack
    concat_in = [np.concatenate(per_core[name], axis=0) for name in in_names]
    concat_zero = [np.zeros((N_CORES * z.shape[0],) + z.shape[1:], z.dtype)
                   for z in zero_outs]
    out_arrs = sharded(*concat_in, *concat_zero)
    outs = {}
    for i, name in enumerate(out_names):
        full = np.asarray(out_arrs[i]).reshape(
            (N_CORES,) + tuple(out_avals[i].shape))
        outs[name] = full
    return outs


def kernel(x, W1, W2, W3, a1, a2, a3):
    exec_pack = _get_exec(float(a1), float(a2), float(a3))
    per_core = prepare_inputs(x, W1, W2, W3)
    outs = run_sharded(per_core, exec_pack)
    out_t = outs["outT"]                     # [N_CORES, 4, BC]
    out = np.empty((B, D3), dtype=np.float32)
    for c in range(N_CORES):
        out[c * BC:(c + 1) * BC] = out_t[c].T
    return out

